# revision 3
# baseline (speedup 1.0000x reference)
"""Trainium2 Bass kernel for the differentiable gaussian-splat renderer.

Full-input contract: kernel(**inputs) takes the unsharded inputs and returns
the full [2*16, 3, 32, 32] output.

Math (per pose):
    cam = positions @ R.T + t ;  pj = (fx*cam_x/cam_z + cx, fy*cam_y/cam_z + cy)
    w[n, p] = op_n * exp(-0.5*((px-ax_n)^2 + (py-ay_n)^2)/s_n^2)
    img = (w.T @ colors) / (w.T @ 1 + 1e-8)

The gaussian weight is separable: w = op * wx[n,px] * wy[n,py].

Sharding: 8 cores = 2 poses x 4 gaussian shards (1024 gaussians each). Each
core evaluates the FULL 128x128 image partial accumulators (num, den) for its
shard; the host sums the 4 shard partials per pose (the all-reduce step) and
normalizes. Projection, quadratic-coefficient construction and the 3-piece
bf16 splits are all done on the host (cheap O(N) numpy); the device receives
pre-transposed coefficient packs and runs only: 4 arg matmuls -> 2 exps ->
X build -> 8 accumulation matmuls -> DMA out.

The exp argument g*(q-ax)^2 (q centered at 64) is a matmul of per-gaussian
quadratic coefficients [g, -2*g*ax, g*ax^2] (each split into 3 bf16 pieces,
exact to ~24 bits) against a block-diagonal pixel basis [q^2(hi,lo), q, 1].
px and py share one basis (both are 0..127 centered). The accumulation
matmul runs in bf16 (tolerance 2e-2 leaves ample margin).
"""

import numpy as np

H = 128
W = 128
FX = 120.0
FY = 120.0
N = 4096
NPOSE = 2
NSHARD = 4           # gaussian shards (cores per pose)
NG = N // NSHARD     # 1024 gaussians per core
NCHUNK = NG // 128   # 8 chunks of 128 gaussians
F32 = np.float32

_CACHE = {}


def _quat2mat(q):
    q = np.asarray(q, dtype=np.float64)
    q = q / np.linalg.norm(q)
    w, x, y, z = q
    return np.array([
        [1 - 2 * (y * y + z * z), 2 * (x * y - z * w), 2 * (x * z + y * w)],
        [2 * (x * y + z * w), 1 - 2 * (x * x + z * z), 2 * (y * z - x * w)],
        [2 * (x * z - y * w), 2 * (y * z + x * w), 1 - 2 * (x * x + y * y)],
    ])


def _build_program():
    """Build the SPMD Bass/Tile program (same program on every core)."""
    import concourse.bacc as bacc
    import concourse.tile as tile
    import concourse.mybir as mybir
    from contextlib import ExitStack

    dt = mybir.dt.float32
    bf = mybir.dt.bfloat16
    nc = bacc.Bacc()

    # ---- DRAM I/O (per-core shapes) ----
    # inp cols: 0:128 coefT_x | 128:256 coefT_y | 256:288 ca (c-major: 8c+j)
    inp_d = nc.dram_tensor("inp", [128, 288], bf, kind="ExternalInput").ap()
    # shared block-diag basis: rows 16j+r active in cols 128j+q
    bas_d = nc.dram_tensor("bas", [128, 1024], bf, kind="ExternalInput").ap()
    out_d = nc.dram_tensor("out", [128, 512], dt, kind="ExternalOutput").ap()

    mult = mybir.AluOpType.mult
    EXP = mybir.ActivationFunctionType.Exp

    with tile.TileContext(nc) as tc, ExitStack() as ctx:
        const = ctx.enter_context(tc.tile_pool(name="const", bufs=1))
        work = ctx.enter_context(tc.tile_pool(name="work", bufs=1))
        ppo = ctx.enter_context(tc.tile_pool(name="ppo", bufs=1, space="PSUM"))
        ppx = ctx.enter_context(tc.tile_pool(name="ppx", bufs=1, space="PSUM"))
        ppy = ctx.enter_context(tc.tile_pool(name="ppy", bufs=1, space="PSUM"))

        po = ppo.tile([128, 512], dt, tag="po")
        pax = ppx.tile([128, 1024], dt, tag="pax")
        pay = ppy.tile([128, 1024], dt, tag="pay")

        inp = const.tile([128, 288], bf, tag="inp")
        nc.sync.dma_start(out=inp[:], in_=inp_d)
        bas = const.tile([128, 1024], bf, tag="bas")
        nc.sync.dma_start(out=bas[:], in_=bas_d)

        coefT_x = inp[:, 0:128]
        coefT_y = inp[:, 128:256]
        ca = inp[:, 256:288]

        # ---- ca_wide [128, (j,c,px)=4096]: broadcast ca over px (early; only
        #      depends on the small input DMA) ----
        caw = const.tile([128, 4096], bf, tag="caw")
        caw_v = caw[:].rearrange("p (j c x) -> p j c x", j=8, c=4)
        for c in range(4):
            eng = nc.vector if c % 2 == 0 else nc.gpsimd
            src = ca[:, 8 * c:8 * c + 8].unsqueeze(2).broadcast_to([128, 8, 128])
            eng.tensor_copy(out=caw_v[:, :, c, :], in_=src)

        # ---- arg matmuls + exp (split in halves for pipelining) ----
        wx = const.tile([128, 1024], bf, tag="wx")
        wy = const.tile([128, 1024], bf, tag="wy")
        for h in range(2):
            nc.tensor.matmul(pax[:, 512 * h:512 * h + 512], lhsT=coefT_x,
                             rhs=bas[:, 512 * h:512 * h + 512],
                             start=True, stop=True)
        for h in range(2):
            nc.tensor.matmul(pay[:, 512 * h:512 * h + 512], lhsT=coefT_y,
                             rhs=bas[:, 512 * h:512 * h + 512],
                             start=True, stop=True)
        for h in range(2):
            nc.scalar.activation(out=wx[:, 512 * h:512 * h + 512],
                                 in_=pax[:, 512 * h:512 * h + 512], func=EXP)
        for h in range(2):
            nc.scalar.activation(out=wy[:, 512 * h:512 * h + 512],
                                 in_=pay[:, 512 * h:512 * h + 512], func=EXP)

        # ---- X [128, (j,c,px)=4096] = ca_wide * wx (broadcast over c) ----
        X = const.tile([128, 4096], bf, tag="X")
        X_v = X[:].rearrange("p (j c x) -> p j c x", j=8, c=4)
        wx_v = wx[:].rearrange("p (j x) -> p j x", j=8)
        for h in range(2):
            j0, j1 = 4 * h, 4 * h + 4
            for c in range(4):
                eng = nc.gpsimd if c == 3 else nc.vector
                eng.tensor_tensor(out=X_v[:, j0:j1, c, :],
                                  in0=wx_v[:, j0:j1, :],
                                  in1=caw_v[:, j0:j1, c, :], op=mult)

        # ---- main accumulation: po[py, (c,px)] += wy_j.T @ X_j ----
        for j in range(NCHUNK):
            nc.tensor.matmul(po[:], lhsT=wy[:, 128 * j:128 * j + 128],
                             rhs=X[:, 512 * j:512 * j + 512],
                             start=(j == 0), stop=(j == NCHUNK - 1))

        # ---- copy partials to SBUF and DMA out ----
        ob = work.tile([128, 512], dt, tag="ob")
        nc.vector.tensor_copy(out=ob[:, 0:256], in_=po[:, 0:256])
        nc.scalar.activation(out=ob[:, 256:512], in_=po[:, 256:512],
                             func=mybir.ActivationFunctionType.Copy)
        nc.sync.dma_start(out=out_d, in_=ob[:])

    nc.compile()
    return nc


def _split3(v, bf):
    """Split f64 array v into 3 bf16 pieces summing to ~24-bit accuracy."""
    s1 = v.astype(bf)
    s2 = (v - s1.astype(np.float64)).astype(bf)
    s3 = (v - s1.astype(np.float64) - s2.astype(np.float64)).astype(bf)
    return s1, s2, s3


def _host_prep(positions, colors, opacities, scales, qvec, tvec):
    """Build the 8 per-core input maps (all projection/coef math on host)."""
    import ml_dtypes
    bf = ml_dtypes.bfloat16

    positions = np.asarray(positions, dtype=np.float64)
    colors = np.asarray(colors, dtype=np.float64)
    opacities = np.asarray(opacities, dtype=np.float64)
    scales = np.asarray(scales, dtype=np.float64)
    qvec = np.asarray(qvec, dtype=np.float64)
    tvec = np.asarray(tvec, dtype=np.float64)

    gneg = -0.5 / (scales[:, 0] ** 2)            # [N]
    cav = np.concatenate([colors * opacities, opacities], axis=1)  # [N,4]

    # shared block-diagonal basis [128, 1024]
    q = np.arange(128.0) - 64.0
    p2 = q * q
    p2h = p2.astype(bf)
    p2l = (p2 - p2h.astype(np.float64)).astype(bf)
    basis = np.zeros((128, 1024), bf)
    for j in range(8):
        r0, c0 = 16 * j, 128 * j
        for r in (0, 2, 4):
            basis[r0 + r, c0:c0 + 128] = p2h
            basis[r0 + r + 1, c0:c0 + 128] = p2l
        for r in (6, 7, 8):
            basis[r0 + r, c0:c0 + 128] = q.astype(bf)
        for r in (9, 10, 11):
            basis[r0 + r, c0:c0 + 128] = 1.0

    in_maps = []
    for p in range(NPOSE):
        R = _quat2mat(qvec[p])
        t = tvec[p]
        A = np.zeros((3, 4))
        A[0, :3] = FX * R[0]
        A[0, 3] = FX * t[0]
        A[1, :3] = FY * R[1]
        A[1, 3] = FY * t[1]
        A[2, :3] = R[2]
        A[2, 3] = t[2]
        cam = positions @ A[:, :3].T + A[:, 3]   # [N,3]
        ax = cam[:, 0] / cam[:, 2]               # centered (cx=64 -> q=px-64)
        ay = cam[:, 1] / cam[:, 2]

        for s in range(NSHARD):
            g0 = s * NG
            sl = slice(g0, g0 + NG)

            def coefT(a_c):
                """[128, 128] bf16: rows 16j+r, cols m (gaussian j*128+m)."""
                gg = gneg[sl].reshape(NCHUNK, 128)
                b = (-2.0 * gneg[sl] * a_c[sl]).reshape(NCHUNK, 128)
                cc = (gneg[sl] * a_c[sl] * a_c[sl]).reshape(NCHUNK, 128)
                out = np.zeros((128, 128), bf)
                for j in range(NCHUNK):
                    a1, a2, a3 = _split3(gg[j], bf)
                    out[16 * j + 0, :] = a1
                    out[16 * j + 1, :] = a1
                    out[16 * j + 2, :] = a2
                    out[16 * j + 3, :] = a2
                    out[16 * j + 4, :] = a3
                    out[16 * j + 5, :] = a3
                    b1, b2, b3 = _split3(b[j], bf)
                    out[16 * j + 6, :] = b1
                    out[16 * j + 7, :] = b2
                    out[16 * j + 8, :] = b3
                    c1, c2, c3 = _split3(cc[j], bf)
                    out[16 * j + 9, :] = c1
                    out[16 * j + 10, :] = c2
                    out[16 * j + 11, :] = c3
                return out

            inp = np.zeros((128, 288), bf)
            inp[:, 0:128] = coefT(ax)
            inp[:, 128:256] = coefT(ay)
            # ca cols c-major: 8c+j
            cv = cav[sl].reshape(NCHUNK, 128, 4)
            for c in range(4):
                inp[:, 256 + 8 * c:256 + 8 * c + 8] = cv[:, :, c].T.astype(bf)
            in_maps.append({"inp": inp, "bas": basis})
    return in_maps


def _assemble(slabs):
    """slabs: 8 x [128, 512] partials -> [NPOSE*16, 3, 32, 32] output."""
    out = []
    for p in range(NPOSE):
        acc = np.zeros((128, 512), np.float64)
        for s in range(NSHARD):
            acc += slabs[p * NSHARD + s].astype(np.float64)
        den = acc[:, 384:512] + 1e-8             # [py, px]
        img = np.empty((H, W, 3), np.float64)
        for c in range(3):
            img[:, :, c] = acc[:, 128 * c:128 * c + 128] / den
        tiles = img.reshape(H * W, 3).reshape(16, 1024, 3)
        tiles = tiles.transpose(0, 2, 1).reshape(16, 3, 32, 32)
        out.append(tiles)
    return np.concatenate(out, axis=0).astype(F32)


def kernel(positions, colors, opacities, scales, qvec, tvec, _trace=False):
    from concourse.bass_utils import run_bass_kernel_spmd

    if "nc" not in _CACHE:
        _CACHE["nc"] = _build_program()
    nc = _CACHE["nc"]

    in_maps = _host_prep(positions, colors, opacities, scales, qvec, tvec)
    res = run_bass_kernel_spmd(nc, in_maps, core_ids=list(range(8)),
                               trace=_trace)
    slabs = [np.asarray(res.results[c]["out"]) for c in range(8)]
    out = _assemble(slabs)
    if _trace:
        _CACHE["last_result"] = res
    return out


# revision 7
# speedup vs baseline: 1.0928x; 1.0928x over previous
"""Trainium2 Bass kernel for the differentiable gaussian-splat renderer.

Full-input contract: kernel(**inputs) takes the unsharded inputs and returns
the full [2*16, 3, 32, 32] output.

Math (per pose):
    cam = positions @ R.T + t ;  pj = (fx*cam_x/cam_z + cx, fy*cam_y/cam_z + cy)
    w[n, p] = op_n * exp(-0.5*((px-ax_n)^2 + (py-ay_n)^2)/s_n^2)
    img = (w.T @ colors) / (w.T @ 1 + 1e-8)

The gaussian weight is separable: w = op * wx[n,px] * wy[n,py].

Sharding: 8 cores = 2 poses x 4 gaussian shards (1024 gaussians each). Each
core evaluates the FULL 128x128 image partial accumulators (num, den) for its
shard; the host sums the 4 shard partials per pose (the all-reduce step) and
normalizes. Projection, quadratic-coefficient construction and the 3-piece
bf16 splits are all done on the host (cheap O(N) numpy); the device receives
pre-transposed coefficient packs and runs only: 4 arg matmuls -> 2 exps ->
X build -> 8 accumulation matmuls -> DMA out.

The exp argument g*(q-ax)^2 (q centered at 64) is a matmul of per-gaussian
quadratic coefficients [g, -2*g*ax, g*ax^2] (each split into 3 bf16 pieces,
exact to ~24 bits) against a block-diagonal pixel basis [q^2(hi,lo), q, 1].
px and py share one basis (both are 0..127 centered). The accumulation
matmul runs in bf16 (tolerance 2e-2 leaves ample margin).
"""

import numpy as np

H = 128
W = 128
FX = 120.0
FY = 120.0
N = 4096
NPOSE = 2
NSHARD = 4           # gaussian shards (cores per pose)
NG = N // NSHARD     # 1024 gaussians per core
NCHUNK = NG // 128   # 8 chunks of 128 gaussians
F32 = np.float32

_CACHE = {}


def _quat2mat(q):
    q = np.asarray(q, dtype=np.float64)
    q = q / np.linalg.norm(q)
    w, x, y, z = q
    return np.array([
        [1 - 2 * (y * y + z * z), 2 * (x * y - z * w), 2 * (x * z + y * w)],
        [2 * (x * y + z * w), 1 - 2 * (x * x + z * z), 2 * (y * z - x * w)],
        [2 * (x * z - y * w), 2 * (y * z + x * w), 1 - 2 * (x * x + y * y)],
    ])


def _build_program():
    """Build the SPMD Bass/Tile program (same program on every core)."""
    import concourse.bacc as bacc
    import concourse.tile as tile
    import concourse.mybir as mybir
    from contextlib import ExitStack

    dt = mybir.dt.float32
    bf = mybir.dt.bfloat16
    nc = bacc.Bacc()

    # ---- DRAM I/O (per-core shapes) ----
    # inp cols: 0:128 coefT_x | 128:256 coefT_y
    inp_d = nc.dram_tensor("inp", [128, 256], bf, kind="ExternalInput").ap()
    # shared block-diag basis: rows 16j+r active in cols 128j+q
    bas_d = nc.dram_tensor("bas", [128, 1024], bf, kind="ExternalInput").ap()
    # pre-broadcast colors*opacity: caw[m, (j,c,px)] = ca_c(j,m)
    caw_d = nc.dram_tensor("caw", [128, 4096], bf, kind="ExternalInput").ap()
    out_d = nc.dram_tensor("out", [128, 512], dt, kind="ExternalOutput").ap()

    mult = mybir.AluOpType.mult
    EXP = mybir.ActivationFunctionType.Exp

    with tile.TileContext(nc) as tc, ExitStack() as ctx:
        const = ctx.enter_context(tc.tile_pool(name="const", bufs=1))
        work = ctx.enter_context(tc.tile_pool(name="work", bufs=1))
        ppo = ctx.enter_context(tc.tile_pool(name="ppo", bufs=1, space="PSUM"))
        ppx = ctx.enter_context(tc.tile_pool(name="ppx", bufs=1, space="PSUM"))
        ppy = ctx.enter_context(tc.tile_pool(name="ppy", bufs=1, space="PSUM"))

        po = ppo.tile([128, 512], dt, tag="po")
        pax = ppx.tile([128, 1024], dt, tag="pax")
        pay = ppy.tile([128, 1024], dt, tag="pay")

        inp = const.tile([128, 256], bf, tag="inp")
        nc.sync.dma_start(out=inp[:], in_=inp_d)
        bas = const.tile([128, 1024], bf, tag="bas")
        nc.sync.dma_start(out=bas[:], in_=bas_d)
        caw = const.tile([128, 4096], bf, tag="caw")
        nc.sync.dma_start(out=caw[:], in_=caw_d)
        caw_v = caw[:].rearrange("p (j c x) -> p j c x", j=8, c=4)

        coefT_x = inp[:, 0:128]
        coefT_y = inp[:, 128:256]

        # ---- arg matmuls + exp (split in halves for pipelining) ----
        wx = const.tile([128, 1024], bf, tag="wx")
        wy = const.tile([128, 1024], bf, tag="wy")
        for h in range(2):
            nc.tensor.matmul(pax[:, 512 * h:512 * h + 512], lhsT=coefT_x,
                             rhs=bas[:, 512 * h:512 * h + 512],
                             start=True, stop=True)
        for h in range(2):
            nc.tensor.matmul(pay[:, 512 * h:512 * h + 512], lhsT=coefT_y,
                             rhs=bas[:, 512 * h:512 * h + 512],
                             start=True, stop=True)
        # scalar queue order: wx halves first so X build can start early
        for h in range(2):
            nc.scalar.activation(out=wx[:, 512 * h:512 * h + 512],
                                 in_=pax[:, 512 * h:512 * h + 512], func=EXP)
        for h in range(2):
            nc.scalar.activation(out=wy[:, 512 * h:512 * h + 512],
                                 in_=pay[:, 512 * h:512 * h + 512], func=EXP)

        # ---- X [128, (j,c,px)=4096] = ca_wide * wx (broadcast over c);
        #      interleaved with main matmuls in halves ----
        X = const.tile([128, 4096], bf, tag="X")
        X_v = X[:].rearrange("p (j c x) -> p j c x", j=8, c=4)
        wx_v = wx[:].rearrange("p (j x) -> p j x", j=8)
        for h in range(2):
            j0, j1 = 4 * h, 4 * h + 4
            for c in range(4):
                eng = nc.gpsimd if c == 3 else nc.vector
                eng.tensor_tensor(out=X_v[:, j0:j1, c, :],
                                  in0=wx_v[:, j0:j1, :],
                                  in1=caw_v[:, j0:j1, c, :], op=mult)
            # main accumulation for this half: po += wy_j.T @ X_j
            for j in range(4 * h, 4 * h + 4):
                nc.tensor.matmul(po[:], lhsT=wy[:, 128 * j:128 * j + 128],
                                 rhs=X[:, 512 * j:512 * j + 512],
                                 start=(j == 0), stop=(j == NCHUNK - 1))

        # ---- copy partials to SBUF (PSUM is not DMA-able) and DMA out ----
        ob = work.tile([128, 512], dt, tag="ob")
        nc.vector.tensor_copy(out=ob[:, 0:256], in_=po[:, 0:256])
        nc.scalar.activation(out=ob[:, 256:512], in_=po[:, 256:512],
                             func=mybir.ActivationFunctionType.Copy)
        nc.sync.dma_start(out=out_d, in_=ob[:])

    nc.compile()
    return nc


def _split3(v, bf):
    """Split f64 array v into 3 bf16 pieces summing to ~24-bit accuracy."""
    s1 = v.astype(bf)
    s2 = (v - s1.astype(np.float64)).astype(bf)
    s3 = (v - s1.astype(np.float64) - s2.astype(np.float64)).astype(bf)
    return s1, s2, s3


def _host_prep(positions, colors, opacities, scales, qvec, tvec):
    """Build the 8 per-core input maps (all projection/coef math on host)."""
    import ml_dtypes
    bf = ml_dtypes.bfloat16

    positions = np.asarray(positions, dtype=np.float64)
    colors = np.asarray(colors, dtype=np.float64)
    opacities = np.asarray(opacities, dtype=np.float64)
    scales = np.asarray(scales, dtype=np.float64)
    qvec = np.asarray(qvec, dtype=np.float64)
    tvec = np.asarray(tvec, dtype=np.float64)

    gneg = -0.5 / (scales[:, 0] ** 2)            # [N]
    cav = np.concatenate([colors * opacities, opacities], axis=1)  # [N,4]

    # shared block-diagonal basis [128, 1024]
    q = np.arange(128.0) - 64.0
    p2 = q * q
    p2h = p2.astype(bf)
    p2l = (p2 - p2h.astype(np.float64)).astype(bf)
    basis = np.zeros((128, 1024), bf)
    for j in range(8):
        r0, c0 = 16 * j, 128 * j
        for r in (0, 2, 4):
            basis[r0 + r, c0:c0 + 128] = p2h
            basis[r0 + r + 1, c0:c0 + 128] = p2l
        for r in (6, 7, 8):
            basis[r0 + r, c0:c0 + 128] = q.astype(bf)
        for r in (9, 10, 11):
            basis[r0 + r, c0:c0 + 128] = 1.0

    in_maps = []
    for p in range(NPOSE):
        R = _quat2mat(qvec[p])
        t = tvec[p]
        A = np.zeros((3, 4))
        A[0, :3] = FX * R[0]
        A[0, 3] = FX * t[0]
        A[1, :3] = FY * R[1]
        A[1, 3] = FY * t[1]
        A[2, :3] = R[2]
        A[2, 3] = t[2]
        cam = positions @ A[:, :3].T + A[:, 3]   # [N,3]
        ax = cam[:, 0] / cam[:, 2]               # centered (cx=64 -> q=px-64)
        ay = cam[:, 1] / cam[:, 2]

        for s in range(NSHARD):
            g0 = s * NG
            sl = slice(g0, g0 + NG)

            def coefT(a_c):
                """[128, 128] bf16: rows 16j+r, cols m (gaussian j*128+m)."""
                gg = gneg[sl].reshape(NCHUNK, 128)
                b = (-2.0 * gneg[sl] * a_c[sl]).reshape(NCHUNK, 128)
                cc = (gneg[sl] * a_c[sl] * a_c[sl]).reshape(NCHUNK, 128)
                out = np.zeros((128, 128), bf)
                for j in range(NCHUNK):
                    a1, a2, a3 = _split3(gg[j], bf)
                    out[16 * j + 0, :] = a1
                    out[16 * j + 1, :] = a1
                    out[16 * j + 2, :] = a2
                    out[16 * j + 3, :] = a2
                    out[16 * j + 4, :] = a3
                    out[16 * j + 5, :] = a3
                    b1, b2, b3 = _split3(b[j], bf)
                    out[16 * j + 6, :] = b1
                    out[16 * j + 7, :] = b2
                    out[16 * j + 8, :] = b3
                    c1, c2, c3 = _split3(cc[j], bf)
                    out[16 * j + 9, :] = c1
                    out[16 * j + 10, :] = c2
                    out[16 * j + 11, :] = c3
                return out

            inp = np.zeros((128, 256), bf)
            inp[:, 0:128] = coefT(ax)
            inp[:, 128:256] = coefT(ay)
            # caw[m, (j,c,px)] = ca_c(gaussian j*128+m), pre-broadcast over px
            cv = cav[sl].reshape(NCHUNK, 128, 4).astype(bf)   # [j, m, c]
            caw = np.broadcast_to(cv.transpose(1, 0, 2)[:, :, :, None],
                                  (128, NCHUNK, 4, 128)).reshape(128, 4096)
            in_maps.append({"inp": inp, "bas": basis,
                            "caw": np.ascontiguousarray(caw)})
    return in_maps


def _assemble(slabs):
    """slabs: 8 x [128, 512] partials -> [NPOSE*16, 3, 32, 32] output."""
    out = []
    for p in range(NPOSE):
        acc = np.zeros((128, 512), np.float64)
        for s in range(NSHARD):
            acc += slabs[p * NSHARD + s].astype(np.float64)
        den = acc[:, 384:512] + 1e-8             # [py, px]
        img = np.empty((H, W, 3), np.float64)
        for c in range(3):
            img[:, :, c] = acc[:, 128 * c:128 * c + 128] / den
        tiles = img.reshape(H * W, 3).reshape(16, 1024, 3)
        tiles = tiles.transpose(0, 2, 1).reshape(16, 3, 32, 32)
        out.append(tiles)
    return np.concatenate(out, axis=0).astype(F32)


def kernel(positions, colors, opacities, scales, qvec, tvec, _trace=False):
    from concourse.bass_utils import run_bass_kernel_spmd

    if "nc" not in _CACHE:
        _CACHE["nc"] = _build_program()
    nc = _CACHE["nc"]

    in_maps = _host_prep(positions, colors, opacities, scales, qvec, tvec)
    res = run_bass_kernel_spmd(nc, in_maps, core_ids=list(range(8)),
                               trace=_trace)
    slabs = [np.asarray(res.results[c]["out"]) for c in range(8)]
    out = _assemble(slabs)
    if _trace:
        _CACHE["last_result"] = res
    return out
